# revision 20
# baseline (speedup 1.0000x reference)
"""Trainium2 Bass kernel for SlidingWindowAttention3d (3x3x3 window, D8 H56 W56, 8 heads).

Sharding: 8 cores = batch(4) x z-halves(2). Each core owns 12544 tokens
(4 z-planes of 56x56) and receives a z-halo in its input slab; cores are
fully independent (no collectives). One SPMD program for all cores; half 1
processes its z-range mirrored so the z-edge is always plane 0 (edge tiles
simply skip the dz=0 window offsets).

On-chip layout: channels (C=128 = 8 heads x 16 dims) on partitions, tokens on
the free axis.  A 3D window offset (dz,dy,dx) is a linear shift
dz*3136 + dy*56 + dx along the token axis; y/x wrap-arounds read a wrong-but-
finite neighbor and are zeroed in the attention weights afterwards.

DVE 2x perf mode requires 4B-aligned 16-bit operands; odd dx shifts break
that, so k and v are stored twice: the plain buffer (even shifts) and a
one-token-shifted copy (odd shifts land on even offsets of the copy).

Per 448-token tile (8 y-rows; one z-plane covers 7 tiles), per offset j:
  DVE Hadamard q*k_shift (f16 2x) -> PE block-ones reduce (head sums,
  d-duplicated) -> ACT exp(S + rpb[j]) -> bf16 a_j -> boundary memsets ->
  PE block-ones/16 dend accumulate + DVE a_j*v_shift -> PE identity
  accumulate (AV sum).  Epilogue: reciprocal of dend, weight AV, out-proj.
"""

from contextlib import ExitStack

import numpy as np
import ml_dtypes

import concourse.bass as bass
from concourse import bacc
import concourse.mybir as mybir
import concourse.tile as tile
from concourse.bass_utils import run_bass_kernel_spmd

F32 = mybir.dt.float32
F16 = mybir.dt.float16
BF16 = mybir.dt.bfloat16

D, H, W = 8, 56, 56
NH, HD, C = 8, 16, 128
N = D * H * W            # 25088
B = 4
NCORES = 8
PLANE = H * W            # 3136
NOWN = 4 * PLANE         # 12544 tokens per core
PAD = 3360               # z-halo pad (>= 3137+56+1 = 3194), 2*PAD % 448 == 0
NEXT = NOWN + 2 * PAD    # 19264
T = 448                  # tile: 8 y-rows
NT = NOWN // T           # 28 tiles per core
import os as _os
_NT_LIMIT = int(_os.environ.get("K_NT_LIMIT", NT))
NCH = NEXT // T          # 43 prologue chunks
TILES_PER_PLANE = 7
NJ = 27
SSB_TILES = 14           # distinct seq-scale tile patterns (z-edge plane + 1 interior plane)


def _patch_act_tables():
    """Force all ACT funcs onto one table set (natural_log_exp_and_others holds
    Exp/Ln/Square/Identity/Copy) so no per-tile table reloads are emitted."""
    import concourse.hw_specs as hw_specs
    if getattr(hw_specs, "_ant_act_tables_patched", False):
        return
    orig = hw_specs.get_activation_tables

    def patched(module_arch):
        tabs = dict(orig(module_arch))
        keep = "natural_log_exp_and_others"
        if keep in tabs:
            tabs = {k: (v if k == keep else set()) for k, v in tabs.items()}
        return tabs

    hw_specs.get_activation_tables = patched
    bacc.get_activation_tables = patched
    hw_specs._ant_act_tables_patched = True


def _build_nc() -> bass.Bass:
    _patch_act_tables()
    nc = bacc.Bacc("TRN2")

    x_ext = nc.dram_tensor("x_ext", [C, NEXT], F16, kind="ExternalInput")
    w_q = nc.dram_tensor("w_q", [C, C], F16, kind="ExternalInput")
    w_k = nc.dram_tensor("w_k", [C, C], F16, kind="ExternalInput")
    w_v = nc.dram_tensor("w_v", [C, C], F16, kind="ExternalInput")
    w_p = nc.dram_tensor("w_p", [C, C], F16, kind="ExternalInput")
    odup = nc.dram_tensor("odup", [C, C], F16, kind="ExternalInput")
    odup16 = nc.dram_tensor("odup16", [C, C], F16, kind="ExternalInput")
    rpbt = nc.dram_tensor("rpbt", [C, NJ], F32, kind="ExternalInput")
    ident = nc.dram_tensor("ident", [C, C], BF16, kind="ExternalInput")
    kb = nc.dram_tensor("kb", [C, 1], F32, kind="ExternalInput")
    vb = nc.dram_tensor("vb", [C, 1], F32, kind="ExternalInput")
    qb = nc.dram_tensor("qb", [C, 1], F32, kind="ExternalInput")
    pb = nc.dram_tensor("pb", [C, 1], F32, kind="ExternalInput")
    lnqsc = nc.dram_tensor("lnqsc", [C, 1], F32, kind="ExternalInput")
    qbi = nc.dram_tensor("qbi", [C, 1], F32, kind="ExternalInput")
    ssb_in = nc.dram_tensor("ssb", [C, SSB_TILES * T], F16, kind="ExternalInput")
    out = nc.dram_tensor("out", [C, NOWN], F32, kind="ExternalOutput")

    with tile.TileContext(nc) as tc, ExitStack() as ctx:
        singles = ctx.enter_context(tc.tile_pool(name="singles", bufs=1))

        k_ext = singles.tile([C, NEXT], F16, tag="k_ext")
        k_odd = singles.tile([C, NEXT], F16, tag="k_odd")   # k_odd[x] = k_ext[x+1]
        v_ext = singles.tile([C, NEXT], BF16, tag="v_ext")
        v_odd = singles.tile([C, NEXT], BF16, tag="v_odd")  # v_odd[x] = v_ext[x+1]

        sb_wq = singles.tile([C, C], F16, tag="wq")
        sb_wk = singles.tile([C, C], F16, tag="wk")
        sb_wv = singles.tile([C, C], F16, tag="wv")
        sb_wp = singles.tile([C, C], F16, tag="wp")
        sb_od = singles.tile([C, C], F16, tag="od")
        sb_od16 = singles.tile([C, C], F16, tag="od16")
        sb_rpbt = singles.tile([C, NJ], F32, tag="rpbt")
        sb_id = singles.tile([C, C], BF16, tag="id")
        sb_kb = singles.tile([C, 1], F32, tag="kb")
        sb_vb = singles.tile([C, 1], F32, tag="vb")
        sb_qb = singles.tile([C, 1], F32, tag="qb")
        sb_pb = singles.tile([C, 1], F32, tag="pb")
        sb_lnqsc = singles.tile([C, 1], F32, tag="lnqsc")
        sb_qbi = singles.tile([C, 1], F32, tag="qbi")
        sb_ssb = singles.tile([C, SSB_TILES * T], F16, tag="ssb")
        sb_eps = singles.tile([C, 1], F32, tag="eps")

        nc.sync.dma_start(out=sb_wq, in_=w_q[:, :])
        nc.sync.dma_start(out=sb_wk, in_=w_k[:, :])
        nc.sync.dma_start(out=sb_wv, in_=w_v[:, :])
        nc.sync.dma_start(out=sb_wp, in_=w_p[:, :])
        nc.sync.dma_start(out=sb_od, in_=odup[:, :])
        nc.sync.dma_start(out=sb_od16, in_=odup16[:, :])
        nc.sync.dma_start(out=sb_rpbt, in_=rpbt[:, :])
        nc.sync.dma_start(out=sb_id, in_=ident[:, :])
        nc.sync.dma_start(out=sb_kb, in_=kb[:, :])
        nc.sync.dma_start(out=sb_vb, in_=vb[:, :])
        nc.sync.dma_start(out=sb_qb, in_=qb[:, :])
        nc.sync.dma_start(out=sb_pb, in_=pb[:, :])
        nc.sync.dma_start(out=sb_lnqsc, in_=lnqsc[:, :])
        nc.sync.dma_start(out=sb_qbi, in_=qbi[:, :])
        nc.sync.dma_start(out=sb_ssb, in_=ssb_in[:, :])
        nc.vector.memset(sb_eps, 1e-24)

        # ---------- k / v production (interleaved with main loop) ----------
        def kv_chunk(ch, px, pst):
                c0 = ch * T
                xc = px.tile([C, T], F16, tag="xc")
                nc.sync.dma_start(out=xc, in_=x_ext[:, c0 : c0 + T])

                kp = psum.tile([C, T], F32, tag="smallmm", bufs=4)
                nc.tensor.matmul(kp, lhsT=sb_wk, rhs=xc, start=True, stop=True)
                vp = psum.tile([C, T], F32, tag="smallmm", bufs=4)
                nc.tensor.matmul(vp, lhsT=sb_wv, rhs=xc, start=True, stop=True)

                nc.scalar.activation(
                    out=v_ext[:, c0 : c0 + T], in_=vp,
                    func=mybir.ActivationFunctionType.Identity, bias=sb_vb, scale=1.0,
                )
                if ch == 0:
                    nc.sync.dma_start(out=v_odd[:, 0 : T - 1], in_=v_ext[:, 1:T])
                else:
                    nc.sync.dma_start(
                        out=v_odd[:, c0 - 1 : c0 - 1 + T], in_=v_ext[:, c0 : c0 + T]
                    )

                kpb = pst.tile([C, T], F32, tag="kpb", bufs=2)
                nc.vector.tensor_scalar(
                    out=kpb, in0=kp, scalar1=sb_kb, scalar2=None,
                    op0=mybir.AluOpType.add,
                )
                sqk = pst.tile([C, T], F16, tag="sqk", bufs=2)
                nc.scalar.activation(
                    out=sqk, in_=kp,
                    func=mybir.ActivationFunctionType.Square, bias=sb_kb, scale=1.0,
                )
                ssq = psum.tile([C, T], F32, tag="smallmm", bufs=4)
                nc.tensor.matmul(ssq, lhsT=sb_od, rhs=sqk, start=True, stop=True)
                nrm = pst.tile([C, T], F32, tag="pnorm", bufs=2)
                nc.scalar.activation(
                    out=nrm, in_=ssq,
                    func=mybir.ActivationFunctionType.Ln, bias=sb_eps, scale=1.0,
                )
                rs = pst.tile([C, T], F32, tag="pnorm", bufs=2)
                nc.scalar.activation(
                    out=rs, in_=nrm,
                    func=mybir.ActivationFunctionType.Exp, bias=0.0, scale=-0.5,
                )
                nc.vector.tensor_mul(k_ext[:, c0 : c0 + T], kpb, rs)
                if ch == 0:
                    nc.sync.dma_start(out=k_odd[:, 0 : T - 1], in_=k_ext[:, 1:T])
                else:
                    nc.sync.dma_start(
                        out=k_odd[:, c0 - 1 : c0 - 1 + T], in_=k_ext[:, c0 : c0 + T]
                    )

        # ---------- main loop over 28 tiles ----------
        with (
            tc.tile_pool(name="px", bufs=3) as px,
            tc.tile_pool(name="pst", bufs=3) as pst,
            tc.tile_pool(name="mx", bufs=3) as mx,
            tc.tile_pool(name="mq", bufs=2) as mq,
            tc.tile_pool(name="ma", bufs=6) as ma,
            tc.tile_pool(name="mp", bufs=3) as mp,
            tc.tile_pool(name="mo", bufs=2) as mo,
            tc.tile_pool(name="psum", bufs=1, space="PSUM") as psum,
        ):
            HEAD_CHUNKS = 17
            for ch in range(min(HEAD_CHUNKS, NCH)):
                kv_chunk(ch, px, pst)

            qf_ring = {}

            def qpipe(t):
                n0 = t * T
                e0 = PAD + n0
                xq = mx.tile([C, T], F16, tag="xq")
                nc.sync.dma_start(out=xq, in_=x_ext[:, e0 : e0 + T])
                qp = psum.tile([C, T], F32, tag="smallmm", bufs=4)
                nc.tensor.matmul(qp, lhsT=sb_wq, rhs=xq, start=True, stop=True)
                sqq = mq.tile([C, T], F16, tag="sqq")
                nc.scalar.activation(
                    out=sqq, in_=qp,
                    func=mybir.ActivationFunctionType.Square, bias=sb_qb, scale=1.0,
                )
                ssqq = psum.tile([C, T], F32, tag="smallmm", bufs=4)
                nc.tensor.matmul(ssqq, lhsT=sb_od, rhs=sqq, start=True, stop=True)
                nrmq = mq.tile([C, T], F32, tag="qtmp", bufs=2)
                nc.scalar.activation(
                    out=nrmq, in_=ssqq,
                    func=mybir.ActivationFunctionType.Ln, bias=sb_eps, scale=1.0,
                )
                # rsq = qsc * ssq^-1/2  (ln(qsc) folded into the Exp bias)
                rsq = mq.tile([C, T], F32, tag="qtmp", bufs=2)
                nc.scalar.activation(
                    out=rsq, in_=nrmq,
                    func=mybir.ActivationFunctionType.Exp, bias=sb_lnqsc, scale=-0.5,
                )
                q1 = mq.tile([C, T], F32, tag="qtmp", bufs=2)
                nc.vector.scalar_tensor_tensor(
                    out=q1, in0=qp, scalar=sb_qb, in1=rsq,
                    op0=mybir.AluOpType.add, op1=mybir.AluOpType.mult,
                )
                sst = t if t < SSB_TILES else TILES_PER_PLANE + (t % TILES_PER_PLANE)
                qf = mq.tile([C, T], F16, tag="qf", bufs=3)
                nc.vector.scalar_tensor_tensor(
                    out=qf, in0=q1, scalar=sb_qbi, in1=sb_ssb[:, sst * T : sst * T + T],
                    op0=mybir.AluOpType.add, op1=mybir.AluOpType.mult,
                )
                qf_ring[t] = qf

            def phases(t):
                n0 = t * T
                e0 = PAD + n0
                r = t % TILES_PER_PLANE
                qf = qf_ring.pop(t)

                avacc = psum.tile([C, T], F32, tag="avacc", bufs=1)
                dend = psum.tile([C, T], F32, tag="dend", bufs=1)

                edge = t < TILES_PER_PLANE  # plane 0 = z edge (mirrored half1)
                js = [
                    (dzi * 9 + dyi * 3 + dxi, dzi, dyi, dxi)
                    for dzi in range(3) for dyi in range(3) for dxi in range(3)
                    if not (edge and dzi == 0)
                ]
                first_j, last_j = js[0][0], js[-1][0]

                def stage1(j, dzi, dyi, dxi):
                    # Hadamard + head-sum reduce + exp + boundary masks -> a_j
                    delta = (dzi - 1) * PLANE + (dyi - 1) * W + (dxi - 1)
                    ks = e0 + delta
                    if dxi == 1:
                        ksrc = k_ext[:, ks : ks + T]
                    else:
                        ksrc = k_odd[:, ks - 1 : ks - 1 + T]
                    pj = mp.tile([C, T], F16, tag="pj", bufs=4)
                    nc.vector.tensor_mul(pj, qf, ksrc)
                    sd = psum.tile([C, T], F32, tag="sd", bufs=2)
                    nc.tensor.matmul(sd, lhsT=sb_od, rhs=pj, start=True, stop=True)
                    aj = ma.tile([C, T], BF16, tag="aj", bufs=10)
                    nc.scalar.activation(
                        out=aj, in_=sd,
                        func=mybir.ActivationFunctionType.Exp,
                        bias=sb_rpbt[:, j : j + 1],
                        scale=1.0,
                    )
                    if dyi == 0 and r == 0:
                        nc.vector.memset(aj[:, 0:W], 0.0)
                    if dyi == 2 and r == TILES_PER_PLANE - 1:
                        nc.vector.memset(aj[:, T - W : T], 0.0)
                    av = aj.rearrange("p (rr x) -> p rr x", x=W)
                    if dxi == 0:
                        nc.vector.memset(av[:, :, 0:1], 0.0)
                    if dxi == 2:
                        nc.vector.memset(av[:, :, W - 1 : W], 0.0)
                    return aj

                def stage2(j, dzi, dyi, dxi, aj):
                    # dend accumulate + AV Hadamard + AV accumulate
                    delta = (dzi - 1) * PLANE + (dyi - 1) * W + (dxi - 1)
                    ks = e0 + delta
                    nc.tensor.matmul(
                        dend, lhsT=sb_od16, rhs=aj,
                        start=(j == first_j), stop=(j == last_j),
                    )
                    if dxi == 1:
                        vsrc = v_ext[:, ks : ks + T]
                    else:
                        vsrc = v_odd[:, ks - 1 : ks - 1 + T]
                    avp = mp.tile([C, T], BF16, tag="avp", bufs=4)
                    nc.vector.tensor_mul(avp, aj, vsrc)
                    nc.tensor.matmul(
                        avacc, lhsT=sb_id, rhs=avp,
                        start=(j == first_j), stop=(j == last_j),
                    )

                # one-j software pipeline: emit stage1(j+1) before stage2(j) so
                # the PE FIFO always has an independent matmul ahead of the
                # dend/avacc matmuls that wait on exp/masks.
                prev = None
                for jj in js:
                    aj = stage1(*jj)
                    if prev is not None:
                        stage2(prev[0][0], prev[0][1], prev[0][2], prev[0][3], prev[1])
                    prev = (jj, aj)
                stage2(prev[0][0], prev[0][1], prev[0][2], prev[0][3], prev[1])

                # epilogue
                rd = mo.tile([C, T], F32, tag="epi", bufs=2)
                scr2 = mo.tile([C, T], F32, tag="epi", bufs=2)
                nc.vector.reciprocal_approx_accurate(out=rd, in_=dend, scratch=scr2)
                outt = mo.tile([C, T], F16, tag="outt")
                nc.vector.tensor_mul(outt, avacc, rd)
                projp = psum.tile([C, T], F32, tag="smallmm", bufs=4)
                nc.tensor.matmul(projp, lhsT=sb_wp, rhs=outt, start=True, stop=True)
                osb = mo.tile([C, T], F32, tag="epi", bufs=2)
                nc.scalar.activation(
                    out=osb, in_=projp,
                    func=mybir.ActivationFunctionType.Identity, bias=sb_pb, scale=1.0,
                )
                nc.sync.dma_start(out=out[:, n0 : n0 + T], in_=osb)

            for t in range(_NT_LIMIT):
                ch = HEAD_CHUNKS + t
                if ch < NCH:
                    kv_chunk(ch, px, pst)
                qpipe(t)
                if t >= 2:
                    phases(t - 2)
            for t in range(max(_NT_LIMIT - 2, 0), _NT_LIMIT):
                phases(t)

    nc.finalize()
    return nc


_NC_CACHE: list = []


def _get_nc() -> bass.Bass:
    if not _NC_CACHE:
        _NC_CACHE.append(_build_nc())
    return _NC_CACHE[0]


def _softplus(x):
    return np.log1p(np.exp(x))


def _host_prep(inputs):
    x = np.asarray(inputs["x"], np.float32)          # [B, N, C]
    q_w = np.asarray(inputs["q_w"], np.float32)      # [C, C]
    q_b = np.asarray(inputs["q_b"], np.float32)
    kv_w = np.asarray(inputs["kv_w"], np.float32)    # [2C, C]
    kv_b = np.asarray(inputs["kv_b"], np.float32)
    proj_w = np.asarray(inputs["proj_w"], np.float32)
    proj_b = np.asarray(inputs["proj_b"], np.float32)
    temp = np.asarray(inputs["temperature"], np.float32).reshape(NH)
    qe = np.asarray(inputs["query_embedding"], np.float32).reshape(NH, HD)
    rpb = np.asarray(inputs["rel_pos_bias"], np.float32)  # [NH, 27]

    sp = _softplus(temp)
    qsc = np.repeat(sp, HD).reshape(C, 1).astype(np.float32)
    qbi = (qe * sp[:, None]).reshape(C, 1).astype(np.float32)
    rpb_dup = np.repeat(rpb, HD, axis=0).astype(np.float32)  # [C, 27]

    def span(i, L):
        return 3 - (i == 0) - (i == L - 1)
    z = np.arange(D)[:, None, None]
    y = np.arange(H)[None, :, None]
    xx = np.arange(W)[None, None, :]
    cnt = span(z, D) * span(y, H) * span(xx, W)
    ss_full = np.log(cnt.astype(np.float32)).reshape(N)

    blk = np.zeros((C, C), np.float32)
    for h in range(NH):
        blk[h * HD : (h + 1) * HD, h * HD : (h + 1) * HD] = 1.0

    common = {
        "w_q": q_w.T.astype(np.float16),
        "w_k": kv_w[:C].T.astype(np.float16),
        "w_v": kv_w[C:].T.astype(np.float16),
        "w_p": proj_w.T.astype(np.float16),
        "odup": blk.astype(np.float16),
        "odup16": (blk / 16.0).astype(np.float16),
        "ident": np.eye(C, dtype=np.float32).astype(ml_dtypes.bfloat16),
        "kb": kv_b[:C].reshape(C, 1).astype(np.float32),
        "vb": kv_b[C:].reshape(C, 1).astype(np.float32),
        "qb": q_b.reshape(C, 1).astype(np.float32),
        "pb": proj_b.reshape(C, 1).astype(np.float32),
        "lnqsc": np.log(qsc), "qbi": qbi,
    }

    in_maps = []
    for core in range(NCORES):
        b, half = core // 2, core % 2
        # half 1 processes its z-range mirrored (token order reversed) so the
        # z-edge is always at plane index 0; offset j maps to 26-j.
        if half == 0:
            xb = x[b]
            ss_c = ss_full[:NOWN]
            rpb_c = rpb_dup
        else:
            xb = x[b, ::-1, :]
            ss_c = ss_full[::-1][:NOWN]
            rpb_c = rpb_dup[:, ::-1]

        # seq-scale tile table: tiles 0..6 (z-edge plane) + 7..13 (one
        # interior plane); interior planes repeat.
        ssb_tab = np.ascontiguousarray(
            np.broadcast_to(
                ss_c[: SSB_TILES * T].astype(np.float16)[None, :],
                (C, SSB_TILES * T),
            )
        )

        xt = np.zeros((C, NEXT), np.float16)
        lo, hi = -PAD, NOWN + PAD
        src_lo, src_hi = max(lo, 0), min(hi, N)
        xt[:, src_lo - lo : src_hi - lo] = xb[src_lo:src_hi, :].T.astype(np.float16)

        m = dict(common)
        m["x_ext"] = xt
        m["rpbt"] = np.ascontiguousarray(rpb_c, dtype=np.float32)
        m["ssb"] = ssb_tab
        in_maps.append(m)
    return in_maps


def _gather(res) -> np.ndarray:
    out_full = np.zeros((B, N, C), np.float32)
    for core in range(NCORES):
        b, half = core // 2, core % 2
        o = res.results[core]["out"].T  # [NOWN, C] in (possibly mirrored) order
        if half == 0:
            out_full[b, :NOWN, :] = o
        else:
            out_full[b, NOWN:, :] = o[::-1, :]
    return out_full


def kernel(**inputs) -> np.ndarray:
    in_maps = _host_prep(inputs)
    nc = _get_nc()
    res = run_bass_kernel_spmd(nc, in_maps, core_ids=list(range(NCORES)))
    return _gather(res)


# revision 21
# speedup vs baseline: 1.0010x; 1.0010x over previous
"""Trainium2 Bass kernel for SlidingWindowAttention3d (3x3x3 window, D8 H56 W56, 8 heads).

Sharding: 8 cores = batch(4) x z-halves(2). Each core owns 12544 tokens
(4 z-planes of 56x56) and receives a z-halo in its input slab; cores are
fully independent (no collectives). One SPMD program for all cores; half 1
processes its z-range mirrored so the z-edge is always plane 0 (edge tiles
simply skip the dz=0 window offsets).

On-chip layout: channels (C=128 = 8 heads x 16 dims) on partitions, tokens on
the free axis.  A 3D window offset (dz,dy,dx) is a linear shift
dz*3136 + dy*56 + dx along the token axis; y/x wrap-arounds read a wrong-but-
finite neighbor and are zeroed in the attention weights afterwards.

DVE 2x perf mode requires 4B-aligned 16-bit operands; odd dx shifts break
that, so k and v are stored twice: the plain buffer (even shifts) and a
one-token-shifted copy (odd shifts land on even offsets of the copy).

Per 448-token tile (8 y-rows; one z-plane covers 7 tiles), per offset j:
  DVE Hadamard q*k_shift (f16 2x) -> PE block-ones reduce (head sums,
  d-duplicated) -> ACT exp(S + rpb[j]) -> bf16 a_j -> boundary memsets ->
  PE block-ones/16 dend accumulate + DVE a_j*v_shift -> PE identity
  accumulate (AV sum).  Epilogue: reciprocal of dend, weight AV, out-proj.
"""

from contextlib import ExitStack

import numpy as np
import ml_dtypes

import concourse.bass as bass
from concourse import bacc
import concourse.mybir as mybir
import concourse.tile as tile
from concourse.bass_utils import run_bass_kernel_spmd

F32 = mybir.dt.float32
F16 = mybir.dt.float16
BF16 = mybir.dt.bfloat16

D, H, W = 8, 56, 56
NH, HD, C = 8, 16, 128
N = D * H * W            # 25088
B = 4
NCORES = 8
PLANE = H * W            # 3136
NOWN = 4 * PLANE         # 12544 tokens per core
PAD = 3360               # z-halo pad (>= 3137+56+1 = 3194), 2*PAD % 448 == 0
NEXT = NOWN + 2 * PAD    # 19264
T = 448                  # tile: 8 y-rows
NT = NOWN // T           # 28 tiles per core
import os as _os
_NT_LIMIT = int(_os.environ.get("K_NT_LIMIT", NT))
NCH = NEXT // T          # 43 prologue chunks
TILES_PER_PLANE = 7
NJ = 27
SSB_TILES = 14           # distinct seq-scale tile patterns (z-edge plane + 1 interior plane)


def _patch_act_tables():
    """Force all ACT funcs onto one table set (natural_log_exp_and_others holds
    Exp/Ln/Square/Identity/Copy) so no per-tile table reloads are emitted."""
    import concourse.hw_specs as hw_specs
    if getattr(hw_specs, "_ant_act_tables_patched", False):
        return
    orig = hw_specs.get_activation_tables

    def patched(module_arch):
        tabs = dict(orig(module_arch))
        keep = "natural_log_exp_and_others"
        if keep in tabs:
            tabs = {k: (v if k == keep else set()) for k, v in tabs.items()}
        return tabs

    hw_specs.get_activation_tables = patched
    bacc.get_activation_tables = patched
    hw_specs._ant_act_tables_patched = True


def _build_nc() -> bass.Bass:
    _patch_act_tables()
    nc = bacc.Bacc("TRN2")

    x_ext = nc.dram_tensor("x_ext", [C, NEXT], F16, kind="ExternalInput")
    w_q = nc.dram_tensor("w_q", [C, C], F16, kind="ExternalInput")
    w_k = nc.dram_tensor("w_k", [C, C], F16, kind="ExternalInput")
    w_v = nc.dram_tensor("w_v", [C, C], F16, kind="ExternalInput")
    w_p = nc.dram_tensor("w_p", [C, C], F16, kind="ExternalInput")
    odup = nc.dram_tensor("odup", [C, C], F16, kind="ExternalInput")
    odup16 = nc.dram_tensor("odup16", [C, C], F16, kind="ExternalInput")
    rpbt = nc.dram_tensor("rpbt", [C, NJ], F32, kind="ExternalInput")
    ident = nc.dram_tensor("ident", [C, C], BF16, kind="ExternalInput")
    kb = nc.dram_tensor("kb", [C, 1], F32, kind="ExternalInput")
    vb = nc.dram_tensor("vb", [C, 1], F32, kind="ExternalInput")
    qb = nc.dram_tensor("qb", [C, 1], F32, kind="ExternalInput")
    pb = nc.dram_tensor("pb", [C, 1], F32, kind="ExternalInput")
    lnqsc = nc.dram_tensor("lnqsc", [C, 1], F32, kind="ExternalInput")
    qbi = nc.dram_tensor("qbi", [C, 1], F32, kind="ExternalInput")
    ssb_in = nc.dram_tensor("ssb", [C, SSB_TILES * T], F16, kind="ExternalInput")
    out = nc.dram_tensor("out", [C, NOWN], F32, kind="ExternalOutput")

    with tile.TileContext(nc) as tc, ExitStack() as ctx:
        singles = ctx.enter_context(tc.tile_pool(name="singles", bufs=1))

        k_ext = singles.tile([C, NEXT], F16, tag="k_ext")
        k_odd = singles.tile([C, NEXT], F16, tag="k_odd")   # k_odd[x] = k_ext[x+1]
        v_ext = singles.tile([C, NEXT], BF16, tag="v_ext")
        v_odd = singles.tile([C, NEXT], BF16, tag="v_odd")  # v_odd[x] = v_ext[x+1]

        sb_wq = singles.tile([C, C], F16, tag="wq")
        sb_wk = singles.tile([C, C], F16, tag="wk")
        sb_wv = singles.tile([C, C], F16, tag="wv")
        sb_wp = singles.tile([C, C], F16, tag="wp")
        sb_od = singles.tile([C, C], F16, tag="od")
        sb_od16 = singles.tile([C, C], F16, tag="od16")
        sb_rpbt = singles.tile([C, NJ], F32, tag="rpbt")
        sb_id = singles.tile([C, C], BF16, tag="id")
        sb_kb = singles.tile([C, 1], F32, tag="kb")
        sb_vb = singles.tile([C, 1], F32, tag="vb")
        sb_qb = singles.tile([C, 1], F32, tag="qb")
        sb_pb = singles.tile([C, 1], F32, tag="pb")
        sb_lnqsc = singles.tile([C, 1], F32, tag="lnqsc")
        sb_qbi = singles.tile([C, 1], F32, tag="qbi")
        sb_ssb = singles.tile([C, SSB_TILES * T], F16, tag="ssb")
        sb_eps = singles.tile([C, 1], F32, tag="eps")

        nc.sync.dma_start(out=sb_wq, in_=w_q[:, :])
        nc.sync.dma_start(out=sb_wk, in_=w_k[:, :])
        nc.sync.dma_start(out=sb_wv, in_=w_v[:, :])
        nc.sync.dma_start(out=sb_wp, in_=w_p[:, :])
        nc.sync.dma_start(out=sb_od, in_=odup[:, :])
        nc.sync.dma_start(out=sb_od16, in_=odup16[:, :])
        nc.sync.dma_start(out=sb_rpbt, in_=rpbt[:, :])
        nc.sync.dma_start(out=sb_id, in_=ident[:, :])
        nc.sync.dma_start(out=sb_kb, in_=kb[:, :])
        nc.sync.dma_start(out=sb_vb, in_=vb[:, :])
        nc.sync.dma_start(out=sb_qb, in_=qb[:, :])
        nc.sync.dma_start(out=sb_pb, in_=pb[:, :])
        nc.sync.dma_start(out=sb_lnqsc, in_=lnqsc[:, :])
        nc.sync.dma_start(out=sb_qbi, in_=qbi[:, :])
        nc.sync.dma_start(out=sb_ssb, in_=ssb_in[:, :])
        nc.vector.memset(sb_eps, 1e-24)

        # ---------- k / v production (interleaved with main loop) ----------
        def kv_chunk(ch, px, pst):
                c0 = ch * T
                xc = px.tile([C, T], F16, tag="xc")
                nc.sync.dma_start(out=xc, in_=x_ext[:, c0 : c0 + T])

                kp = psum.tile([C, T], F32, tag="smallmm", bufs=4)
                nc.tensor.matmul(kp, lhsT=sb_wk, rhs=xc, start=True, stop=True)
                vp = psum.tile([C, T], F32, tag="smallmm", bufs=4)
                nc.tensor.matmul(vp, lhsT=sb_wv, rhs=xc, start=True, stop=True)

                nc.scalar.activation(
                    out=v_ext[:, c0 : c0 + T], in_=vp,
                    func=mybir.ActivationFunctionType.Identity, bias=sb_vb, scale=1.0,
                )
                if ch == 0:
                    nc.sync.dma_start(out=v_odd[:, 0 : T - 1], in_=v_ext[:, 1:T])
                else:
                    nc.sync.dma_start(
                        out=v_odd[:, c0 - 1 : c0 - 1 + T], in_=v_ext[:, c0 : c0 + T]
                    )

                kpb = pst.tile([C, T], F32, tag="kpb", bufs=2)
                nc.vector.tensor_scalar(
                    out=kpb, in0=kp, scalar1=sb_kb, scalar2=None,
                    op0=mybir.AluOpType.add,
                )
                sqk = pst.tile([C, T], F16, tag="sqk", bufs=2)
                nc.scalar.activation(
                    out=sqk, in_=kp,
                    func=mybir.ActivationFunctionType.Square, bias=sb_kb, scale=1.0,
                )
                ssq = psum.tile([C, T], F32, tag="smallmm", bufs=4)
                nc.tensor.matmul(ssq, lhsT=sb_od, rhs=sqk, start=True, stop=True)
                nrm = pst.tile([C, T], F32, tag="pnorm", bufs=2)
                nc.scalar.activation(
                    out=nrm, in_=ssq,
                    func=mybir.ActivationFunctionType.Ln, bias=sb_eps, scale=1.0,
                )
                rs = pst.tile([C, T], F32, tag="pnorm", bufs=2)
                nc.scalar.activation(
                    out=rs, in_=nrm,
                    func=mybir.ActivationFunctionType.Exp, bias=0.0, scale=-0.5,
                )
                nc.vector.tensor_mul(k_ext[:, c0 : c0 + T], kpb, rs)
                if ch == 0:
                    nc.sync.dma_start(out=k_odd[:, 0 : T - 1], in_=k_ext[:, 1:T])
                else:
                    nc.sync.dma_start(
                        out=k_odd[:, c0 - 1 : c0 - 1 + T], in_=k_ext[:, c0 : c0 + T]
                    )

        # ---------- main loop over 28 tiles ----------
        with (
            tc.tile_pool(name="px", bufs=2) as px,
            tc.tile_pool(name="pst", bufs=3) as pst,
            tc.tile_pool(name="mx", bufs=2) as mx,
            tc.tile_pool(name="mq", bufs=2) as mq,
            tc.tile_pool(name="ma", bufs=6) as ma,
            tc.tile_pool(name="mp", bufs=3) as mp,
            tc.tile_pool(name="mo", bufs=2) as mo,
            tc.tile_pool(name="psum", bufs=1, space="PSUM") as psum,
        ):
            HEAD_CHUNKS = 17
            for ch in range(min(HEAD_CHUNKS, NCH)):
                kv_chunk(ch, px, pst)

            qf_ring = {}

            def qpipe(t):
                n0 = t * T
                e0 = PAD + n0
                xq = mx.tile([C, T], F16, tag="xq")
                nc.sync.dma_start(out=xq, in_=x_ext[:, e0 : e0 + T])
                qp = psum.tile([C, T], F32, tag="smallmm", bufs=4)
                nc.tensor.matmul(qp, lhsT=sb_wq, rhs=xq, start=True, stop=True)
                sqq = mq.tile([C, T], F16, tag="sqq")
                nc.scalar.activation(
                    out=sqq, in_=qp,
                    func=mybir.ActivationFunctionType.Square, bias=sb_qb, scale=1.0,
                )
                ssqq = psum.tile([C, T], F32, tag="smallmm", bufs=4)
                nc.tensor.matmul(ssqq, lhsT=sb_od, rhs=sqq, start=True, stop=True)
                nrmq = mq.tile([C, T], F32, tag="qtmp", bufs=2)
                nc.scalar.activation(
                    out=nrmq, in_=ssqq,
                    func=mybir.ActivationFunctionType.Ln, bias=sb_eps, scale=1.0,
                )
                # rsq = qsc * ssq^-1/2  (ln(qsc) folded into the Exp bias)
                rsq = mq.tile([C, T], F32, tag="qtmp", bufs=2)
                nc.scalar.activation(
                    out=rsq, in_=nrmq,
                    func=mybir.ActivationFunctionType.Exp, bias=sb_lnqsc, scale=-0.5,
                )
                q1 = mq.tile([C, T], F32, tag="qtmp", bufs=2)
                nc.vector.scalar_tensor_tensor(
                    out=q1, in0=qp, scalar=sb_qb, in1=rsq,
                    op0=mybir.AluOpType.add, op1=mybir.AluOpType.mult,
                )
                sst = t if t < SSB_TILES else TILES_PER_PLANE + (t % TILES_PER_PLANE)
                qf = mq.tile([C, T], F16, tag="qf", bufs=3)
                nc.vector.scalar_tensor_tensor(
                    out=qf, in0=q1, scalar=sb_qbi, in1=sb_ssb[:, sst * T : sst * T + T],
                    op0=mybir.AluOpType.add, op1=mybir.AluOpType.mult,
                )
                qf_ring[t] = qf

            def phases(t):
                n0 = t * T
                e0 = PAD + n0
                r = t % TILES_PER_PLANE
                qf = qf_ring.pop(t)

                avacc = psum.tile([C, T], F32, tag="avacc", bufs=1)
                dend = psum.tile([C, T], F32, tag="dend", bufs=1)

                edge = t < TILES_PER_PLANE  # plane 0 = z edge (mirrored half1)
                js = [
                    (dzi * 9 + dyi * 3 + dxi, dzi, dyi, dxi)
                    for dzi in range(3) for dyi in range(3) for dxi in range(3)
                    if not (edge and dzi == 0)
                ]
                first_j, last_j = js[0][0], js[-1][0]

                def stage1(j, dzi, dyi, dxi):
                    # Hadamard + head-sum reduce + exp + boundary masks -> a_j
                    delta = (dzi - 1) * PLANE + (dyi - 1) * W + (dxi - 1)
                    ks = e0 + delta
                    if dxi == 1:
                        ksrc = k_ext[:, ks : ks + T]
                    else:
                        ksrc = k_odd[:, ks - 1 : ks - 1 + T]
                    pj = mp.tile([C, T], F16, tag="pj", bufs=4)
                    nc.vector.tensor_mul(pj, qf, ksrc)
                    sd = psum.tile([C, T], F32, tag="sd", bufs=2)
                    nc.tensor.matmul(sd, lhsT=sb_od, rhs=pj, start=True, stop=True)
                    aj = ma.tile([C, T], BF16, tag="aj", bufs=8)
                    nc.scalar.activation(
                        out=aj, in_=sd,
                        func=mybir.ActivationFunctionType.Exp,
                        bias=sb_rpbt[:, j : j + 1],
                        scale=1.0,
                    )
                    if dyi == 0 and r == 0:
                        nc.vector.memset(aj[:, 0:W], 0.0)
                    if dyi == 2 and r == TILES_PER_PLANE - 1:
                        nc.vector.memset(aj[:, T - W : T], 0.0)
                    av = aj.rearrange("p (rr x) -> p rr x", x=W)
                    if dxi == 0:
                        nc.vector.memset(av[:, :, 0:1], 0.0)
                    if dxi == 2:
                        nc.vector.memset(av[:, :, W - 1 : W], 0.0)
                    return aj

                def stage2(j, dzi, dyi, dxi, aj):
                    # dend accumulate + AV Hadamard + AV accumulate
                    delta = (dzi - 1) * PLANE + (dyi - 1) * W + (dxi - 1)
                    ks = e0 + delta
                    nc.tensor.matmul(
                        dend, lhsT=sb_od16, rhs=aj,
                        start=(j == first_j), stop=(j == last_j),
                    )
                    if dxi == 1:
                        vsrc = v_ext[:, ks : ks + T]
                    else:
                        vsrc = v_odd[:, ks - 1 : ks - 1 + T]
                    avp = mp.tile([C, T], BF16, tag="avp", bufs=4)
                    nc.vector.tensor_mul(avp, aj, vsrc)
                    nc.tensor.matmul(
                        avacc, lhsT=sb_id, rhs=avp,
                        start=(j == first_j), stop=(j == last_j),
                    )

                # one-j software pipeline: emit stage1(j+1) before stage2(j) so
                # the PE FIFO always has an independent matmul ahead of the
                # dend/avacc matmuls that wait on exp/masks.
                prev = None
                for jj in js:
                    aj = stage1(*jj)
                    if prev is not None:
                        stage2(prev[0][0], prev[0][1], prev[0][2], prev[0][3], prev[1])
                    prev = (jj, aj)
                stage2(prev[0][0], prev[0][1], prev[0][2], prev[0][3], prev[1])

                # epilogue
                rd = mo.tile([C, T], F32, tag="epi", bufs=2)
                scr2 = mo.tile([C, T], F32, tag="epi", bufs=2)
                nc.vector.reciprocal_approx_accurate(out=rd, in_=dend, scratch=scr2)
                outt = mo.tile([C, T], F16, tag="outt")
                nc.vector.tensor_mul(outt, avacc, rd)
                projp = psum.tile([C, T], F32, tag="smallmm", bufs=4)
                nc.tensor.matmul(projp, lhsT=sb_wp, rhs=outt, start=True, stop=True)
                osb = mo.tile([C, T], F32, tag="epi", bufs=2)
                nc.scalar.activation(
                    out=osb, in_=projp,
                    func=mybir.ActivationFunctionType.Identity, bias=sb_pb, scale=1.0,
                )
                nc.sync.dma_start(out=out[:, n0 : n0 + T], in_=osb)

            for t in range(_NT_LIMIT):
                ch = HEAD_CHUNKS + t
                if ch < NCH:
                    kv_chunk(ch, px, pst)
                qpipe(t)
                if t >= 2:
                    phases(t - 2)
            for t in range(max(_NT_LIMIT - 2, 0), _NT_LIMIT):
                phases(t)

    nc.finalize()
    return nc


_NC_CACHE: list = []


def _get_nc() -> bass.Bass:
    if not _NC_CACHE:
        _NC_CACHE.append(_build_nc())
    return _NC_CACHE[0]


def _softplus(x):
    return np.log1p(np.exp(x))


def _host_prep(inputs):
    x = np.asarray(inputs["x"], np.float32)          # [B, N, C]
    q_w = np.asarray(inputs["q_w"], np.float32)      # [C, C]
    q_b = np.asarray(inputs["q_b"], np.float32)
    kv_w = np.asarray(inputs["kv_w"], np.float32)    # [2C, C]
    kv_b = np.asarray(inputs["kv_b"], np.float32)
    proj_w = np.asarray(inputs["proj_w"], np.float32)
    proj_b = np.asarray(inputs["proj_b"], np.float32)
    temp = np.asarray(inputs["temperature"], np.float32).reshape(NH)
    qe = np.asarray(inputs["query_embedding"], np.float32).reshape(NH, HD)
    rpb = np.asarray(inputs["rel_pos_bias"], np.float32)  # [NH, 27]

    sp = _softplus(temp)
    qsc = np.repeat(sp, HD).reshape(C, 1).astype(np.float32)
    qbi = (qe * sp[:, None]).reshape(C, 1).astype(np.float32)
    rpb_dup = np.repeat(rpb, HD, axis=0).astype(np.float32)  # [C, 27]

    def span(i, L):
        return 3 - (i == 0) - (i == L - 1)
    z = np.arange(D)[:, None, None]
    y = np.arange(H)[None, :, None]
    xx = np.arange(W)[None, None, :]
    cnt = span(z, D) * span(y, H) * span(xx, W)
    ss_full = np.log(cnt.astype(np.float32)).reshape(N)

    blk = np.zeros((C, C), np.float32)
    for h in range(NH):
        blk[h * HD : (h + 1) * HD, h * HD : (h + 1) * HD] = 1.0

    common = {
        "w_q": q_w.T.astype(np.float16),
        "w_k": kv_w[:C].T.astype(np.float16),
        "w_v": kv_w[C:].T.astype(np.float16),
        "w_p": proj_w.T.astype(np.float16),
        "odup": blk.astype(np.float16),
        "odup16": (blk / 16.0).astype(np.float16),
        "ident": np.eye(C, dtype=np.float32).astype(ml_dtypes.bfloat16),
        "kb": kv_b[:C].reshape(C, 1).astype(np.float32),
        "vb": kv_b[C:].reshape(C, 1).astype(np.float32),
        "qb": q_b.reshape(C, 1).astype(np.float32),
        "pb": proj_b.reshape(C, 1).astype(np.float32),
        "lnqsc": np.log(qsc), "qbi": qbi,
    }

    in_maps = []
    for core in range(NCORES):
        b, half = core // 2, core % 2
        # half 1 processes its z-range mirrored (token order reversed) so the
        # z-edge is always at plane index 0; offset j maps to 26-j.
        if half == 0:
            xb = x[b]
            ss_c = ss_full[:NOWN]
            rpb_c = rpb_dup
        else:
            xb = x[b, ::-1, :]
            ss_c = ss_full[::-1][:NOWN]
            rpb_c = rpb_dup[:, ::-1]

        # seq-scale tile table: tiles 0..6 (z-edge plane) + 7..13 (one
        # interior plane); interior planes repeat.
        ssb_tab = np.ascontiguousarray(
            np.broadcast_to(
                ss_c[: SSB_TILES * T].astype(np.float16)[None, :],
                (C, SSB_TILES * T),
            )
        )

        xt = np.zeros((C, NEXT), np.float16)
        lo, hi = -PAD, NOWN + PAD
        src_lo, src_hi = max(lo, 0), min(hi, N)
        xt[:, src_lo - lo : src_hi - lo] = xb[src_lo:src_hi, :].T.astype(np.float16)

        m = dict(common)
        m["x_ext"] = xt
        m["rpbt"] = np.ascontiguousarray(rpb_c, dtype=np.float32)
        m["ssb"] = ssb_tab
        in_maps.append(m)
    return in_maps


def _gather(res) -> np.ndarray:
    out_full = np.zeros((B, N, C), np.float32)
    for core in range(NCORES):
        b, half = core // 2, core % 2
        o = res.results[core]["out"].T  # [NOWN, C] in (possibly mirrored) order
        if half == 0:
            out_full[b, :NOWN, :] = o
        else:
            out_full[b, NOWN:, :] = o[::-1, :]
    return out_full


def kernel(**inputs) -> np.ndarray:
    in_maps = _host_prep(inputs)
    nc = _get_nc()
    res = run_bass_kernel_spmd(nc, in_maps, core_ids=list(range(NCORES)))
    return _gather(res)
